# revision 3
# baseline (speedup 1.0000x reference)
"""GAT layer (gnn_message_passing) Trainium2 kernel.

Math (after algebraic simplification of the reference):
  v3 = W3 @ W5[:64];  v4 = W4 @ W5[64:]           # [64]
  s3 = drug_emb @ v3                               # [N_DRUG]
  s4[t] = tail_emb[t] . v4  (computed on the fly from gathered rows)
  Sb = drug_emb @ (rel_emb * (W1 @ 1)).T + sum(b1) # [N_DRUG, N_REL]
  att_e  = leaky_relu(s3[h_e] + s4[t_e])
  p_e    = exp(att_e)            (softmax max-shift dropped: shift-invariant)
  w_e    = p_e * Sb[h_e, r_e]
  U[h]   = sum_e w_e * tail_emb[t_e];  den[h] = sum_e p_e
  neigh  = U / den
  y      = [drug_emb | neigh] @ W2 + b2;  out = batchnorm(y) (training stats)

Sharding: edges sorted by head on the host (index-only preprocessing);
8 cores own disjoint 2500-head ranges, so segment stats complete locally.
Only the 64x2 batchnorm statistics are all-reduced.

Device per core: dma_gather tail rows; per-128-edge block one-hot matmuls
implement the per-edge s3/Sb gathers and the segment reduction in PSUM.
"""

import math

import numpy as np

import concourse.bacc as bacc
import concourse.bass as bass
import concourse.tile as tile
from concourse import mybir
from concourse.bass_utils import run_bass_kernel_spmd

F32 = mybir.dt.float32
I16 = mybir.dt.int16
AF = mybir.ActivationFunctionType
OP = mybir.AluOpType

N_DRUG = 20000
N_TAIL = 20000
N_REL = 64
D = 64
NC = 8
HPC = N_DRUG // NC          # heads per core
WIN = 128                   # heads per window
NWIN = (HPC + WIN - 1) // WIN  # windows per core (20)
DROWS = NWIN * WIN          # padded drug rows per core (2560)
EPS = 1e-5
SLOPE = 0.01
GRP = 8                     # blocks per batched-scalar group
HROW_BLKS = 16              # blocks per hrel-row staging tile
NQ = 1                      # SWDGE queues for dma_gather desc-gen
DMA_SCRATCH = 16384         # SWDGE descriptor-ring carveout (bytes)
GCH = 8                     # blocks per dma_gather call (ring capacity)
USE_GATHER = True           # debug: memset t_tile instead of dma_gather
USE_CC = True               # debug: skip the stats all-reduce


def _build_nc(NBW: int):
    """Build the Bass module. NBW = 128-edge blocks per 128-head window."""
    NB = NWIN * NBW          # blocks per core
    S = NB * 128             # edge slots per core

    nc = bacc.Bacc(None, num_devices=NC, num_swdge_queues=NQ,
                   dynamic_dma_scratch_size=DMA_SCRATCH)

    # ---- I/O ----
    def inp(name, shape, dtype=F32):
        return nc.declare_dram_parameter(name, list(shape), dtype, isOutput=False)

    tail_emb = inp("tail_emb", (N_TAIL, D))
    drug_rows = inp("drug_rows", (DROWS, D))
    rel_emb = inp("rel_emb", (N_REL, D))
    W1 = inp("W1", (D, D))
    W2 = inp("W2", (2 * D, D))
    W3 = inp("W3", (D, D))
    W4 = inp("W4", (D, D))
    W5 = inp("W5", (2 * D, 1))
    b1c = inp("b1c", (D, 1))
    b2r = inp("b2r", (1, D))
    gammac = inp("gammac", (D, 1))
    betac = inp("betac", (D, 1))

    hrelC = inp("hrelC", (128, NB))      # hrel (head - window base) per slot
    relC = inp("relC", (128, NB))
    maskC = inp("maskC", (128, NB))
    hrel_flat = inp("hrel_flat", (S,))   # token-major copy for row loads
    tails16 = inp("tails16", (128, S // 16), I16)  # wrapped idx, 8x replicated

    c_iota128 = inp("c_iota128", (128, 128))  # row p = 0..127
    c_iotaP = inp("c_iotaP", (128, 1))        # col = partition index
    c_iota64 = inp("c_iota64", (128, 64))
    c_id128 = inp("c_id128", (128, 128))
    c_id64 = inp("c_id64", (64, 64))
    c_ones1 = inp("c_ones1", (1, 128))
    c_ones64 = inp("c_ones64", (64, 1))
    c_ones128 = inp("c_ones128", (128, 1))

    out_rows = nc.declare_dram_parameter("out_rows", [DROWS, D], F32, isOutput=True)
    out_dbg = nc.declare_dram_parameter("out_dbg", [D, 4], F32, isOutput=True)

    # collective bounce buffers
    cc_in = nc.dram_tensor("cc_in", [D, 2], F32)
    cc_out = nc.dram_tensor("cc_out", [D, 2], F32, addr_space="Shared")

    with tile.TileContext(nc) as tc:
        cst = tc.alloc_tile_pool(name="cst", bufs=1)
        big = tc.alloc_tile_pool(name="big", bufs=1)
        sb = tc.alloc_tile_pool(name="sb", bufs=2)
        ohp = tc.alloc_tile_pool(name="ohp", bufs=GRP + 2)
        wtp = tc.alloc_tile_pool(name="wtp", bufs=2)
        tgp = tc.alloc_tile_pool(name="tgp", bufs=2)
        hrp = tc.alloc_tile_pool(name="hrp", bufs=2)
        grp = tc.alloc_tile_pool(name="grp", bufs=2)
        ps = tc.alloc_tile_pool(name="ps", bufs=2, space="PSUM")
        psG = tc.alloc_tile_pool(name="psG", bufs=2, space="PSUM")
        psU = tc.alloc_tile_pool(name="psU", bufs=2, space="PSUM")
        psS = tc.alloc_tile_pool(name="psS", bufs=1, space="PSUM")

        # absorber: first DVE instruction after the init barrier must carry
        # no data wait (compact DVE structs have a single wait slot).
        dve0 = cst.tile([128, 1], F32, tag="dve0")
        nc.vector.memset(dve0[:], 0.0)

        def mm(out, lhsT, rhs, start=True, stop=True, is_transpose=None):
            return nc.tensor.matmul(out, lhsT, rhs, start=start, stop=stop,
                                    is_transpose=is_transpose,
                                    skip_group_check=True)

        import bass_rust as _br

        def dep(a, b):
            _br.add_dep_helper(a.ins, b.ins, sync=True, reason="wait-routing")

        def load(pool, src_ap, shape, dtype=F32, name=None):
            t = pool.tile(list(shape), dtype, tag=name)
            nc.sync.dma_start(out=t[:], in_=src_ap)
            return t

        # ---- constants into SBUF ----
        iota128 = load(cst, c_iota128[:, :], (128, 128), name="iota128")
        iotaP = load(cst, c_iotaP[:, :], (128, 1), name="iotaP")
        iota64 = load(cst, c_iota64[:, :], (128, 64), name="iota64")
        id128 = load(cst, c_id128[:, :], (128, 128), name="id128")
        id64 = load(cst, c_id64[:, :], (64, 64), name="id64")
        ones1 = load(cst, c_ones1[:, :], (1, 128), name="ones1")
        ones64 = load(cst, c_ones64[:, :], (64, 1), name="ones64")
        ones128 = load(cst, c_ones128[:, :], (128, 1), name="ones128")

        hrelC_t = load(big, hrelC[:, :], (128, NB), name="hrelC")
        relC_t = load(big, relC[:, :], (128, NB), name="relC")
        maskC_t = load(big, maskC[:, :], (128, NB), name="maskC")
        tails_t = load(big, tails16[:, :], (128, S // 16), I16, name="tails")

        w1t = load(cst, W1[:, :], (64, 64), name="w1")
        w2a = load(cst, W2[0:64, :], (64, 64), name="w2a")
        w2b = load(cst, W2[64:128, :], (64, 64), name="w2b")
        w3t = load(cst, W3[:, :], (64, 64), name="w3")
        w4t = load(cst, W4[:, :], (64, 64), name="w4")
        w5a = load(cst, W5[0:64, :], (64, 1), name="w5a")
        w5b = load(cst, W5[64:128, :], (64, 1), name="w5b")
        relt = load(cst, rel_emb[:, :], (64, 64), name="relt")
        b1col = load(cst, b1c[:, :], (64, 1), name="b1col")
        b2row = load(cst, b2r[:, :], (1, 64), name="b2row")
        gcol = load(cst, gammac[:, :], (64, 1), name="gcol")
        bcol = load(cst, betac[:, :], (64, 1), name="bcol")

        # ---- phase 1: weight folding ----
        def transpose_to(pool, src_t, k, m, name):
            # src [k, m] -> dst [m, k]  (PE transpose via identity)
            pst = ps.tile([m, k], F32, tag="ps")
            ident = id64 if k == 64 else id128
            mm(pst[:], src_t[:], ident[:, 0:k], is_transpose=True)
            dst = pool.tile([m, k], F32, tag=name)
            nc.scalar.copy(dst[:], pst[:])
            return dst

        w3T = transpose_to(cst, w3t, 64, 64, "w3T")
        w4T = transpose_to(cst, w4t, 64, 64, "w4T")
        w1T = transpose_to(cst, w1t, 64, 64, "w1T")
        relT = transpose_to(cst, relt, 64, 64, "relT")

        def mm_to_sbuf(pool, lhsT, rhs, m, n, name):
            pst = ps.tile([m, n], F32, tag="ps")
            mm(pst[:], lhsT, rhs)
            dst = pool.tile([m, n], F32, tag=name)
            nc.scalar.copy(dst[:], pst[:])
            return dst

        v3 = mm_to_sbuf(cst, w3T[:], w5a[:], 64, 1, "v3")      # [64,1]
        v4 = mm_to_sbuf(cst, w4T[:], w5b[:], 64, 1, "v4")      # [64,1]
        w1s = mm_to_sbuf(cst, w1T[:], ones64[:], 64, 1, "w1s")  # [64,1]
        b1s = mm_to_sbuf(cst, b1col[:], ones64[:], 1, 1, "b1s")  # [1,1]

        # v4 as a [128, 64] broadcast tile
        psv4r = ps.tile([1, 64], F32, tag="ps")
        mm(psv4r[:], v4[:], id64[:], is_transpose=True)
        v4row = cst.tile([1, 64], F32, tag="v4row")
        nc.scalar.copy(v4row[:], psv4r[:])
        v4tile = cst.tile([128, 64], F32, tag="v4tile")
        nc.gpsimd.partition_broadcast(v4tile[:], v4row[:])

        b1s_tile = cst.tile([128, 1], F32, tag="b1stile")
        nc.gpsimd.partition_broadcast(b1s_tile[:], b1s[:])

        b2tile = cst.tile([128, 64], F32, tag="b2tile")
        nc.gpsimd.partition_broadcast(b2tile[:], b2row[:])

        # M_T = rel_emb.T * w1s (per-partition scale)
        MT = cst.tile([64, 64], F32, tag="MT")
        nc.vector.tensor_scalar(MT[:], relT[:], w1s[:], None, OP.mult)

        # DVE fences: advance DVE's clock past the preload DMAs and the
        # gpsimd broadcasts so hot-loop TensorScalarPtr ops (1 wait slot)
        # never need more than one embedded wait.
        for fi, ft in enumerate((iota128, iotaP, iota64, hrelC_t, relC_t,
                                 maskC_t, v4tile, b2tile, gcol, bcol)):
            np_ = ft.shape[0]
            fj = cst.tile([np_, 1], F32, tag=f"fj{fi}")
            nc.vector.tensor_copy(fj[:], ft[0:np_, 0:1])
        fj16 = cst.tile([128, 1], I16, tag="fj16")
        nc.vector.tensor_copy(fj16[:], tails_t[:, 0:1])

        # ---- phase 2: per-window drug prep ----
        SW = cst.tile([128, NWIN, 65], F32, tag="SW")   # [Sb | s3] per window
        drugTs = []
        for w in range(NWIN):
            dchunk = sb.tile([128, 64], F32, tag="dchunk")
            nc.sync.dma_start(out=dchunk[:], in_=drug_rows[w * 128:(w + 1) * 128, :])
            psDT = ps.tile([64, 128], F32, tag="ps")
            mm(psDT[:], dchunk[:], id128[:], is_transpose=True)
            dT = cst.tile([64, 128], F32, tag=f"drugT{w}")
            nc.scalar.copy(dT[:], psDT[:])
            drugTs.append(dT)
            psSb = ps.tile([128, 64], F32, tag="ps")
            mm(psSb[:], dT[:], MT[:])
            nc.scalar.activation(SW[:, w, 0:64], psSb[:], AF.Identity,
                                 bias=b1s_tile[:], scale=1.0)
            psS3 = ps.tile([128, 1], F32, tag="ps")
            mm(psS3[:], dT[:], v3[:])
            nc.scalar.copy(SW[:, w, 64:65], psS3[:])

        # ---- phase 3: edge pass ----
        neigh = cst.tile([128, NWIN, 64], F32, tag="neigh")

        n_hrow = (NB + HROW_BLKS - 1) // HROW_BLKS
        for w in range(NWIN):
            # gather this window's tail rows: [128, NBW, 64]
            t_tile = tgp.tile([128, NBW, 64], F32, tag="tgath")
            nidx = NBW * 128
            if USE_GATHER:
                gat_i = None
                for g0 in range(0, NBW, GCH):
                    gn = min(GCH, NBW - g0) * 128
                    io = (w * NBW + g0) * 8
                    gat_i = nc.gpsimd.dma_gather(
                        out_ap=t_tile[:, g0:g0 + gn // 128, :],
                        in_ap=tail_emb[:, :],
                        idxs_ap=tails_t[0:16, io:io + gn // 16],
                        num_idxs=gn,
                        num_idxs_reg=gn,
                        elem_size=64,
                        queue_num=w % NQ,
                    )
            else:
                gat_i = nc.vector.memset(t_tile[:], 0.125)

            pU = psU.tile([128, 65], F32, tag="pU")

            for j0 in range(0, NBW, GRP):
                g = min(GRP, NBW - j0)
                s3g = grp.tile([128, GRP], F32, tag="s3g")
                s4g = grp.tile([128, GRP], F32, tag="s4g")
                svg = grp.tile([128, GRP], F32, tag="svg")
                wt8 = wtp.tile([128, GRP, 65], F32, tag="wt8")
                ohs = []
                for jj in range(g):
                    j = j0 + jj
                    b = w * NBW + j
                    # hrel row staging for the broadcast matmul
                    if b % HROW_BLKS == 0:
                        hrow = hrp.tile([1, HROW_BLKS * 128], F32, tag="hrow")
                        hb = min(b + HROW_BLKS, NB) * 128
                        nc.sync.dma_start(
                            out=hrow[0:1, 0:hb - b * 128],
                            in_=hrel_flat[b * 128:hb][None, :])
                    psA = ps.tile([128, 128], F32, tag="ps")
                    co = (b % HROW_BLKS) * 128
                    mm(psA[:], ones1[:], hrow[0:1, co:co + 128])
                    if b == 0:
                        shA = sb.tile([128, 1], F32, tag="shA")
                        nc.vector.tensor_copy(shA[:], psA[:, 0:1])
                    ohT = sb.tile([128, 128], F32, tag="ohT")
                    nc.vector.tensor_scalar(ohT[:], psA[:], iotaP[:], None,
                                            OP.is_equal)
                    pG = psG.tile([128, 65], F32, tag="pG")
                    pg_i = mm(pG[:], ohT[:], SW[:, w, :])
                    # ohr/oh are regular single-wait ops; route the PE (pG)
                    # and DMA (gather) waits through them so the stt ops
                    # below only need their mandatory DVE self-wait.
                    ohr = sb.tile([128, 64], F32, tag="ohr")
                    ohr_i = nc.vector.tensor_scalar(ohr[:], iota64[:],
                                                    relC_t[:, b:b + 1], None,
                                                    OP.is_equal)
                    dep(ohr_i, pg_i)
                    oh = ohp.tile([128, 128], F32, tag="oh")
                    oh_i = nc.vector.tensor_scalar(oh[:], iota128[:],
                                                   hrelC_t[:, b:b + 1], None,
                                                   OP.is_equal)
                    dep(oh_i, gat_i)
                    ohs.append(oh)
                    junk = sb.tile([128, 64], F32, tag="junk")
                    nc.vector.scalar_tensor_tensor(
                        out=junk[:], in0=ohr[:], scalar=0.0, in1=pG[:, 0:64],
                        op0=OP.bypass, op1=OP.mult,
                        accum_out=svg[:, jj:jj + 1])
                    junk2 = sb.tile([128, 64], F32, tag="junk2")
                    nc.vector.scalar_tensor_tensor(
                        out=junk2[:], in0=t_tile[:, j, :], scalar=0.0,
                        in1=v4tile[:], op0=OP.bypass, op1=OP.mult,
                        accum_out=s4g[:, jj:jj + 1])
                    nc.scalar.copy(s3g[:, jj:jj + 1], pG[:, 64:65])

                # batched scalar pipeline for the group. Shield copies first:
                # absorb the ACT (s3g) wait on a regular DVE op so the
                # TensorTensor adds below carry at most one wait each.
                shld = sb.tile([128, 1], F32, tag="shld")
                nc.vector.tensor_copy(shld[:], s3g[:, g - 1:g])
                attg = grp.tile([128, GRP], F32, tag="attg")
                nc.vector.tensor_tensor(out=attg[:, 0:g], in0=s3g[:, 0:g],
                                        in1=s4g[:, 0:g], op=OP.add)
                nc.scalar.activation(attg[:, 0:g], attg[:, 0:g], AF.Lrelu,
                                     bias=0.0, scale=1.0, alpha=SLOPE)
                nc.scalar.activation(attg[:, 0:g], attg[:, 0:g], AF.Exp)
                pmg = grp.tile([128, GRP], F32, tag="pmg")
                nc.vector.tensor_tensor(out=pmg[:, 0:g], in0=attg[:, 0:g],
                                        in1=maskC_t[:, w * NBW + j0:
                                                    w * NBW + j0 + g], op=OP.mult)
                wg = grp.tile([128, GRP], F32, tag="wg")
                nc.vector.tensor_tensor(out=wg[:, 0:g], in0=pmg[:, 0:g],
                                        in1=svg[:, 0:g], op=OP.mult)
                # scaled tail rows + p column
                for jj in range(g):
                    j = j0 + jj
                    nc.vector.tensor_scalar(wt8[:, jj, 0:64], t_tile[:, j, :],
                                            wg[:, jj:jj + 1], None, OP.mult)
                nc.vector.tensor_copy(wt8[:, 0:g, 64], pmg[:, 0:g])
                for jj in range(g):
                    j = j0 + jj
                    mm(pU[:], ohs[jj][:], wt8[:, jj, :],
                       start=(j == 0), stop=(j == NBW - 1))

            # window reduction -> neigh
            dsafe = sb.tile([128, 1], F32, tag="dsafe")
            nc.vector.tensor_scalar(dsafe[:], pU[:, 64:65], 1e-30, None, OP.add)
            recip = sb.tile([128, 1], F32, tag="recip")
            nc.vector.reciprocal(recip[:], dsafe[:])
            nc.vector.tensor_scalar(neigh[:, w, :], pU[:, 0:64], recip[:], None,
                                    OP.mult)

        # ---- phase 4: output head + batchnorm ----
        ybuf = cst.tile([128, NWIN, 64], F32, tag="ybuf")
        pStat0 = psS.tile([64, 1], F32, tag="pStat0")
        pStat1 = psS.tile([64, 1], F32, tag="pStat1")
        for w in range(NWIN):
            psNT = ps.tile([64, 128], F32, tag="ps")
            mm(psNT[:], neigh[:, w, :], id128[:], is_transpose=True)
            nT = sb.tile([64, 128], F32, tag="nT")
            nc.scalar.copy(nT[:], psNT[:])
            pY = ps.tile([128, 64], F32, tag="ps")
            mm(pY[:], drugTs[w][:], w2a[:], start=True, stop=False)
            mm(pY[:], nT[:], w2b[:], start=False, stop=True)
            nc.vector.tensor_tensor(out=ybuf[:, w, :], in0=pY[:], in1=b2tile[:],
                                    op=OP.add)
            sq = sb.tile([128, 64], F32, tag="sq")
            nc.scalar.square(sq[:], ybuf[:, w, :])
            mm(pStat0[:], ybuf[:, w, :], ones128[:],
               start=(w == 0), stop=(w == NWIN - 1))
            mm(pStat1[:], sq[:], ones128[:],
               start=(w == 0), stop=(w == NWIN - 1))

        statsb = sb.tile([64, 2], F32, tag="statsb")
        nc.scalar.copy(statsb[:, 0:1], pStat0[:])
        nc.scalar.copy(statsb[:, 1:2], pStat1[:])
        nc.sync.dma_start(out=cc_in[:, :], in_=statsb[:])
        if USE_CC:
            nc.gpsimd.collective_compute(
                "AllReduce", OP.add, replica_groups=[list(range(NC))],
                ins=[cc_in[:, :]], outs=[cc_out[:, :]])
        else:
            nc.sync.dma_start(out=cc_out[:, :], in_=cc_in[:, :])
        statsg = sb.tile([64, 2], F32, tag="statsg")
        nc.sync.dma_start(out=statsg[:], in_=cc_out[:, :])
        fjs = sb.tile([64, 1], F32, tag="fjs")
        nc.vector.tensor_copy(fjs[:], statsg[:, 0:1])
        nc.sync.dma_start(out=out_dbg[:, 0:2], in_=statsb[:])
        nc.sync.dma_start(out=out_dbg[:, 2:4], in_=statsg[:])

        mean = sb.tile([64, 1], F32, tag="mean")
        nc.vector.tensor_scalar(mean[:], statsg[:, 0:1], 1.0 / N_DRUG, None, OP.mult)
        ex2 = sb.tile([64, 1], F32, tag="ex2")
        nc.vector.tensor_scalar(ex2[:], statsg[:, 1:2], 1.0 / N_DRUG, None, OP.mult)
        msq = sb.tile([64, 1], F32, tag="msq")
        nc.vector.tensor_tensor(out=msq[:], in0=mean[:], in1=mean[:], op=OP.mult)
        var = sb.tile([64, 1], F32, tag="var")
        nc.vector.tensor_tensor(out=var[:], in0=ex2[:], in1=msq[:], op=OP.subtract)
        vare = sb.tile([64, 1], F32, tag="vare")
        nc.vector.tensor_scalar(vare[:], var[:], EPS, None, OP.add)
        sd = sb.tile([64, 1], F32, tag="sd")
        nc.scalar.activation(sd[:], vare[:], AF.Sqrt)
        rstd = sb.tile([64, 1], F32, tag="rstd")
        nc.vector.reciprocal(rstd[:], sd[:])
        scalec = sb.tile([64, 1], F32, tag="scalec")
        nc.vector.tensor_tensor(out=scalec[:], in0=gcol[:], in1=rstd[:], op=OP.mult)
        tmp = sb.tile([64, 1], F32, tag="tmp")
        nc.vector.tensor_tensor(out=tmp[:], in0=mean[:], in1=scalec[:], op=OP.mult)
        shiftc = sb.tile([64, 1], F32, tag="shiftc")
        nc.vector.tensor_tensor(out=shiftc[:], in0=bcol[:], in1=tmp[:],
                                op=OP.subtract)

        def col_to_tile(col, name):
            pst = ps.tile([1, 64], F32, tag="ps")
            mm(pst[:], col[:], id64[:], is_transpose=True)
            row = sb.tile([1, 64], F32, tag=name + "r")
            nc.scalar.copy(row[:], pst[:])
            t = cst.tile([128, 64], F32, tag=name)
            nc.gpsimd.partition_broadcast(t[:], row[:])
            return t

        scale_t = col_to_tile(scalec, "scalet")
        shift_t = col_to_tile(shiftc, "shiftt")
        for fi, ft in enumerate((scale_t, shift_t)):
            fjt = sb.tile([128, 1], F32, tag=f"fjt{fi}")
            nc.vector.tensor_copy(fjt[:], ft[:, 0:1])

        for w in range(NWIN):
            o1 = sb.tile([128, 64], F32, tag="o1")
            nc.vector.tensor_tensor(out=o1[:], in0=ybuf[:, w, :], in1=scale_t[:],
                                    op=OP.mult)
            o2 = sb.tile([128, 64], F32, tag="o2")
            nc.vector.tensor_tensor(out=o2[:], in0=o1[:], in1=shift_t[:], op=OP.add)
            nc.sync.dma_start(out=out_rows[w * 128:(w + 1) * 128, :], in_=o2[:])

        for p in (psS, psU, psG, ps, grp, hrp, tgp, wtp, ohp, sb, big, cst):
            p.release()

    nc.finalize()
    return nc


def _host_prep(DKG):
    """Sort edges by head, shard by head range, build per-core slot arrays."""
    heads = np.asarray(DKG[:, 0], dtype=np.int64)
    tails = np.asarray(DKG[:, 1], dtype=np.int64)
    rels = np.asarray(DKG[:, 2], dtype=np.int64)

    order = np.argsort(heads, kind="stable")
    hs, ts, rs = heads[order], tails[order], rels[order]

    core_lo = np.searchsorted(hs, HPC * np.arange(NC), side="left")
    core_hi = np.searchsorted(hs, HPC * (np.arange(NC) + 1), side="left")

    # window edge counts -> NBW
    winb = np.searchsorted(hs, WIN * np.arange(NC * NWIN), side="left")
    wine = np.searchsorted(hs, WIN * (np.arange(NC * NWIN) + 1), side="left")
    maxw = int((wine - winb).max())
    NBW = max(1, (maxw + 127) // 128)
    NB = NWIN * NBW
    S = NB * 128

    per_core = []
    for c in range(NC):
        lo, hi = core_lo[c], core_hi[c]
        ch, ct, cr = hs[lo:hi], ts[lo:hi], rs[lo:hi]
        hrel = np.zeros(S, np.float32)
        rel = np.zeros(S, np.float32)
        mask = np.zeros(S, np.float32)
        tail = np.zeros(S, np.int64)
        base = c * HPC
        for w in range(NWIN):
            wl = np.searchsorted(ch, base + w * WIN, side="left")
            wh = np.searchsorted(ch, base + (w + 1) * WIN, side="left")
            n = wh - wl
            o = w * NBW * 128
            hrel[o:o + n] = (ch[wl:wh] - base - w * WIN).astype(np.float32)
            rel[o:o + n] = cr[wl:wh].astype(np.float32)
            mask[o:o + n] = 1.0
            tail[o:o + n] = ct[wl:wh]
        hrelC = hrel.reshape(NB, 128).T.copy()
        relC = rel.reshape(NB, 128).T.copy()
        maskC = mask.reshape(NB, 128).T.copy()
        t16 = tail.reshape(S // 16, 16).T.astype(np.int16)          # [16, S/16]
        t16r = np.tile(t16, (8, 1)).copy()                          # [128, S/16]
        per_core.append(dict(hrelC=hrelC, relC=relC, maskC=maskC,
                             hrel_flat=hrel, tails16=t16r))
    return NBW, per_core


def prepare(X, DKG, drug_emb, rel_emb, tail_emb, W1, b1, W2, b2, gamma, beta,
            W3, W4, W5):
    f = np.float32
    NBW, per_core = _host_prep(np.asarray(DKG))
    nc = _build_nc(NBW)

    consts = dict(
        c_iota128=np.broadcast_to(np.arange(128, dtype=f), (128, 128)).copy(),
        c_iotaP=np.arange(128, dtype=f).reshape(128, 1).copy(),
        c_iota64=np.broadcast_to(np.arange(64, dtype=f), (128, 64)).copy(),
        c_id128=np.eye(128, dtype=f),
        c_id64=np.eye(64, dtype=f),
        c_ones1=np.ones((1, 128), f),
        c_ones64=np.ones((64, 1), f),
        c_ones128=np.ones((128, 1), f),
    )
    weights = dict(
        tail_emb=np.asarray(tail_emb, f),
        rel_emb=np.asarray(rel_emb, f),
        W1=np.asarray(W1, f), W2=np.asarray(W2, f), W3=np.asarray(W3, f),
        W4=np.asarray(W4, f), W5=np.asarray(W5, f),
        b1c=np.asarray(b1, f).reshape(D, 1),
        b2r=np.asarray(b2, f).reshape(1, D),
        gammac=np.asarray(gamma, f).reshape(D, 1),
        betac=np.asarray(beta, f).reshape(D, 1),
    )
    de = np.asarray(drug_emb, f)
    in_maps = []
    for c in range(NC):
        dr = np.zeros((DROWS, D), f)
        dr[:HPC] = de[c * HPC:(c + 1) * HPC]
        m = dict(weights)
        m.update(consts)
        m["drug_rows"] = dr
        pc = per_core[c]
        m["hrelC"] = pc["hrelC"]
        m["relC"] = pc["relC"]
        m["maskC"] = pc["maskC"]
        m["hrel_flat"] = pc["hrel_flat"]
        m["tails16"] = pc["tails16"]
        in_maps.append(m)
    return nc, in_maps


def kernel(X, DKG, drug_emb, rel_emb, tail_emb, W1, b1, W2, b2, gamma, beta,
           W3, W4, W5):
    X = np.asarray(X)
    nc, in_maps = prepare(X, DKG, drug_emb, rel_emb, tail_emb, W1, b1, W2, b2,
                          gamma, beta, W3, W4, W5)

    res = run_bass_kernel_spmd(nc, in_maps, core_ids=list(range(NC)))
    global LAST_RESULT
    LAST_RESULT = res
    out = np.concatenate([res.results[c]["out_rows"][:HPC] for c in range(NC)],
                         axis=0)
    return out, X


LAST_RESULT = None

